# revision 22
# baseline (speedup 1.0000x reference)
"""MatchFilter (graph-pair cross-attention + gated segment sum) on 8 trn2 cores.

Math per graph pair b (reference):
    S = L_b @ R_b^T                      [nl, nr]
    P_row = softmax(S, axis=1);  P_col = softmax(S, axis=0)
    wl_i = sigmoid(<L_i, (P_row @ R)_i>) = sigmoid( (sum_j E_ij S_ij) / (sum_j E_ij) )
    wr_j analogously from S^T.
    out_l[b] = sum_i wl_i L_i ;  out_r[b] = sum_j wr_j R_j

Key identity: <L_i, right_atten_i> = sum_j P_row[i,j] * S[i,j] — so the
second attention matmul is never materialized; only row stats of S (and S^T)
are needed.  sigmoid(z) = 0.5 + 0.5*tanh(z/2) keeps ScalarE on one LUT set
(Exp and Tanh share `exp_and_others`).

Sharding: 64 pairs -> 8 cores x 8 pairs, fully local (data parallel over
pairs).  Host pre-swizzles per-core fp16 inputs in both natural ([node, d])
and d-major ([d, node]) layouts so the device does zero transposes.
"""

import os
import numpy as np
from contextlib import ExitStack

import concourse.bass as bass
import concourse.bacc as bacc
import concourse.tile as tile
from concourse import mybir
from concourse.bass_utils import run_bass_kernel_spmd

N_CORES = 8
B = 64            # graph pairs
D = 256           # embedding dim
DC = D // 128     # d-chunks for the 128-deep contraction
PAIRS_PER_CORE = B // N_CORES
# exp(S - EXP_SHIFT): keeps e^S and its row-sums inside f32 range for
# S ~ N(0, 256) (|S| <~ 90).  Max-subtraction is unnecessary because the
# z = t/s ratio is shift-invariant.
EXP_SHIFT = 32.0

LAST_RESULT = None  # BassKernelResults of the most recent run (for test.py)
LAST_TIMING = {}
LAST_IN_MAPS = []

_NC_CACHE = {}


def _build_bass(pairs: int, p: int):
    """Per-core program: `pairs` graph pairs, each padded to `p` nodes/side."""
    f16, f32 = mybir.dt.float16, mybir.dt.float32
    AF = mybir.ActivationFunctionType
    ALU = mybir.AluOpType
    nl = p // 128  # row-chunks per pair side

    nc = bacc.Bacc("TRN2", target_bir_lowering=False, debug=False,
                   num_devices=N_CORES)
    l_nat = nc.dram_tensor("l_nat", [128, nl, pairs, D], f16, kind="ExternalInput").ap()
    r_nat = nc.dram_tensor("r_nat", [128, nl, pairs, D], f16, kind="ExternalInput").ap()
    lt = nc.dram_tensor("lt", [128, DC, pairs, p], f16, kind="ExternalInput").ap()
    rt = nc.dram_tensor("rt", [128, DC, pairs, p], f16, kind="ExternalInput").ap()
    mask = nc.dram_tensor("mask", [128, pairs * (p // 128), 16], f16,
                          kind="ExternalInput").ap()
    out_l = nc.dram_tensor("out_l", [pairs, D], f32, kind="ExternalOutput").ap()
    out_r = nc.dram_tensor("out_r", [pairs, D], f32, kind="ExternalOutput").ap()

    with tile.TileContext(nc) as tc, ExitStack() as ctx:
        sb = ctx.enter_context(tc.tile_pool(name="sb", bufs=1))
        work = ctx.enter_context(tc.tile_pool(name="work", bufs=4))
        stat = ctx.enter_context(tc.tile_pool(name="stat", bufs=1))
        psum = ctx.enter_context(tc.tile_pool(name="psum", bufs=3, space="PSUM"))
        outp = ctx.enter_context(tc.tile_pool(name="outp", bufs=1, space="PSUM"))

        l_sb = sb.tile([128, nl, pairs, D], f16, tag="l_sb")
        r_sb = sb.tile([128, nl, pairs, D], f16, tag="r_sb")
        lt_sb = sb.tile([128, DC, pairs, p], f16, tag="lt_sb")
        rt_sb = sb.tile([128, DC, pairs, p], f16, tag="rt_sb")
        # split loads so pair-0 compute can start before the tail arrives
        nsplit = 2
        step = pairs // nsplit if pairs % nsplit == 0 else pairs
        for s0 in range(0, pairs, step):
            sl = slice(s0, s0 + step)
            nc.sync.dma_start(out=lt_sb[:, :, sl, :], in_=lt[:, :, sl, :])
            nc.sync.dma_start(out=rt_sb[:, :, sl, :], in_=rt[:, :, sl, :])
        for s0 in range(0, pairs, step):
            sl = slice(s0, s0 + step)
            nc.sync.dma_start(out=l_sb[:, :, sl, :], in_=l_nat[:, :, sl, :])
            nc.sync.dma_start(out=r_sb[:, :, sl, :], in_=r_nat[:, :, sl, :])

        mask_sb = stat.tile([128, pairs * nl, 16], f16, tag="mask_sb")
        nc.sync.dma_start(out=mask_sb, in_=mask)
        neg_shift = stat.tile([128, 1], f32, tag="neg_shift")
        nc.vector.memset(neg_shift, -EXP_SHIFT)

        nstat = pairs * nl
        # stats padded to 32B stride so every tiny accum write is 32B-aligned
        sL = stat.tile([128, nstat, 8], f32, tag="sL")
        tL = stat.tile([128, nstat, 8], f32, tag="tL")
        sR = stat.tile([128, nstat, 8], f32, tag="sR")
        tR = stat.tile([128, nstat, 8], f32, tag="tR")

        for b in range(pairs):
            for rc in range(nl):  # row-chunk of the left side (S) / right side (ST)
                k = b * nl + rc
                s_ps = psum.tile([128, p], f32, tag="s")
                st_ps = psum.tile([128, p], f32, tag="st")
                for c in range(DC):
                    nc.tensor.matmul(
                        s_ps, lhsT=lt_sb[:, c, b, rc * 128:(rc + 1) * 128],
                        rhs=rt_sb[:, c, b, :], start=(c == 0), stop=(c == DC - 1))
                for c in range(DC):
                    nc.tensor.matmul(
                        st_ps, lhsT=rt_sb[:, c, b, rc * 128:(rc + 1) * 128],
                        rhs=lt_sb[:, c, b, :], start=(c == 0), stop=(c == DC - 1))
                e = work.tile([128, p], f32, tag="e")
                et = work.tile([128, p], f32, tag="et")
                nc.scalar.activation(out=e, in_=s_ps, func=AF.Exp,
                                     bias=neg_shift, scale=1.0,
                                     accum_out=sL[:, k, 0:1])
                nc.scalar.activation(out=et, in_=st_ps, func=AF.Exp,
                                     bias=neg_shift, scale=1.0,
                                     accum_out=sR[:, k, 0:1])
                scr = work.tile([128, p], f32, tag="scr")
                scrt = work.tile([128, p], f32, tag="scrt")
                nc.vector.scalar_tensor_tensor(
                    out=scr, in0=e, scalar=1.0, in1=s_ps,
                    op0=ALU.mult, op1=ALU.mult, accum_out=tL[:, k, 0:1])
                nc.vector.scalar_tensor_tensor(
                    out=scrt, in0=et, scalar=1.0, in1=st_ps,
                    op0=ALU.mult, op1=ALU.mult, accum_out=tR[:, k, 0:1])

        # gates: w = 0.5 + 0.5*tanh( (t/s) / 2 ) == sigmoid(t/s); the gate for
        # chunk k is splatted onto block-diagonal weights via a host-sent
        # one-hot mask so the final segment sums are [K=128, M=pairs, N=D]
        # accumulating matmuls with output base partition 0.
        gates = {}
        for side, (s_t, t_t) in (("l", (sL, tL)), ("r", (sR, tR))):
            rs = stat.tile([128, nstat], f32, tag=f"rs_{side}")
            z = stat.tile([128, nstat], f32, tag=f"z_{side}")
            h = stat.tile([128, nstat], f32, tag=f"h_{side}")
            w = stat.tile([128, nstat], f32, tag=f"w_{side}")
            wblk = stat.tile([128, nstat, 16], f16, tag=f"wblk_{side}")
            nc.vector.reciprocal(out=rs, in_=s_t[:, :, 0])
            nc.vector.tensor_mul(z, t_t[:, :, 0], rs)
            nc.scalar.activation(out=h, in_=z, func=AF.Tanh, scale=0.5)
            nc.vector.tensor_scalar(out=w, in0=h, scalar1=0.5, scalar2=0.5,
                                    op0=ALU.mult, op1=ALU.add)
            for k in range(nstat):
                nc.vector.tensor_scalar(out=wblk[:, k, :], in0=mask_sb[:, k, :],
                                        scalar1=w[:, k:k + 1], scalar2=None,
                                        op0=ALU.mult)
            gates[side] = wblk

        outl_ps = outp.tile([pairs, D], f32, tag="outl")
        outr_ps = outp.tile([pairs, D], f32, tag="outr")
        for b in range(pairs):
            for rc in range(nl):
                k = b * nl + rc
                nc.tensor.matmul(outl_ps,
                                 lhsT=gates["l"][:, k, 0:pairs],
                                 rhs=l_sb[:, rc, b, :],
                                 start=(k == 0), stop=(k == nstat - 1))
                nc.tensor.matmul(outr_ps,
                                 lhsT=gates["r"][:, k, 0:pairs],
                                 rhs=r_sb[:, rc, b, :],
                                 start=(k == 0), stop=(k == nstat - 1))

        outl_sb = stat.tile([pairs, D], f32, tag="outl_sb")
        outr_sb = stat.tile([pairs, D], f32, tag="outr_sb")
        nc.vector.tensor_copy(out=outl_sb, in_=outl_ps)
        nc.vector.tensor_copy(out=outr_sb, in_=outr_ps)
        nc.sync.dma_start(out=out_l, in_=outl_sb)
        nc.sync.dma_start(out=out_r, in_=outr_sb)

    nc.compile()
    return nc


def _bench_exec(nc, in_maps, reps):
    """Min wall time of the cached jitted 8-core NEFF dispatch (inputs
    pre-sharded on device; excludes jax tracing and input upload)."""
    import time as _time
    import jax
    import jax.numpy as jnp
    from jax.sharding import Mesh, PartitionSpec, NamedSharding
    from jax.experimental.shard_map import shard_map
    from concourse import bass2jax
    from concourse.bass2jax import _bass_exec_p

    n_cores = len(in_maps)
    part_name = nc.partition_id_tensor.name if nc.partition_id_tensor else None
    in_names, out_names, out_avals = [], [], []
    for alloc in nc.m.functions[0].allocations:
        if not isinstance(alloc, mybir.MemoryLocationSet):
            continue
        name = alloc.memorylocations[0].name
        if alloc.kind == "ExternalInput":
            if name != part_name:
                in_names.append(name)
        elif alloc.kind == "ExternalOutput":
            out_names.append(name)
            out_avals.append(jax.core.ShapedArray(
                tuple(alloc.tensor_shape), mybir.dt.np(alloc.dtype)))
    n_params = len(in_names)
    all_in_names = in_names + out_names
    if part_name is not None:
        all_in_names = all_in_names + [part_name]

    def _body(*args):
        operands = list(args)
        if part_name is not None:
            operands.append(bass2jax.partition_id_tensor())
        return tuple(_bass_exec_p.bind(
            *operands, out_avals=tuple(out_avals), in_names=tuple(all_in_names),
            out_names=tuple(out_names), lowering_input_output_aliases=(),
            sim_require_finite=True, sim_require_nnan=True, nc=nc))

    devices = jax.devices()[:n_cores]
    mesh = Mesh(np.asarray(devices), ("core",))
    spec = PartitionSpec("core")
    fn = jax.jit(shard_map(_body, mesh=mesh,
                           in_specs=(spec,) * (n_params + len(out_names)),
                           out_specs=(spec,) * len(out_names)),
                 keep_unused=True)
    sharding = NamedSharding(mesh, spec)
    dev_ins = [jax.device_put(
        np.concatenate([np.asarray(m[name]) for m in in_maps], axis=0), sharding)
        for name in in_names]
    dev_zeros = [jax.device_put(
        np.zeros((n_cores * a.shape[0], *a.shape[1:]), a.dtype), sharding)
        for a in out_avals]
    fn(*dev_ins, *dev_zeros)[0].block_until_ready()  # warm compile
    best = float("inf")
    for _ in range(reps):
        t0 = _time.perf_counter()
        outs = fn(*dev_ins, *dev_zeros)
        for o in outs:
            o.block_until_ready()
        best = min(best, _time.perf_counter() - t0)
    return best


def _noop_baseline(reps):
    """Min wall time of a near-empty program through the same run path —
    estimates the host/axon dispatch overhead included in kernel_wall_s."""
    import time as _time
    if "noop" not in _NC_CACHE:
        nc = bacc.Bacc("TRN2", target_bir_lowering=False, debug=False,
                       num_devices=N_CORES)
        x = nc.dram_tensor("x", [128, 16], mybir.dt.float32,
                           kind="ExternalInput").ap()
        y = nc.dram_tensor("y", [128, 16], mybir.dt.float32,
                           kind="ExternalOutput").ap()
        with tile.TileContext(nc) as tc, ExitStack() as ctx:
            pool = ctx.enter_context(tc.tile_pool(name="p", bufs=1))
            t = pool.tile([128, 16], mybir.dt.float32)
            nc.sync.dma_start(out=t, in_=x)
            nc.sync.dma_start(out=y, in_=t)
        nc.compile()
        _NC_CACHE["noop"] = nc
    nc = _NC_CACHE["noop"]
    ins = [{"x": np.zeros((128, 16), np.float32)} for _ in range(N_CORES)]
    return _bench_exec(nc, ins, reps)


def sim_time_ns(in_map, pairs, p):
    """CoreSim cost-model time for one core's program (ns)."""
    from concourse import bass_interp
    key = (pairs, p)
    if key not in _NC_CACHE:
        _NC_CACHE[key] = _build_bass(*key)
    sim = bass_interp.CoreSim(_NC_CACHE[key])
    for name, arr in in_map.items():
        sim.tensor(name)[:] = arr
    sim.simulate()
    return int(sim.time)


def _pack_side(emb, seg_id, p):
    """[N, D] ragged -> [B, p, D] zero-padded f32 (no-copy reshape if uniform)."""
    counts = np.bincount(seg_id, minlength=B)
    if (counts == p).all():
        return emb.reshape(B, p, D)
    out = np.zeros((B, p, D), emb.dtype)
    offs = np.concatenate([[0], np.cumsum(counts)])
    for g in range(B):
        out[g, :counts[g]] = emb[offs[g]:offs[g + 1]]
    return out


def kernel(left_graph_emb, right_graph_emb, left_x_batch, right_x_batch):
    global LAST_RESULT
    L = np.ascontiguousarray(np.asarray(left_graph_emb, dtype=np.float32))
    R = np.ascontiguousarray(np.asarray(right_graph_emb, dtype=np.float32))
    lb = np.asarray(left_x_batch).astype(np.int64)
    rb = np.asarray(right_x_batch).astype(np.int64)

    maxseg = max(int(np.bincount(lb, minlength=B).max()),
                 int(np.bincount(rb, minlength=B).max()))
    p = max(128, -(-maxseg // 128) * 128)  # pad width, multiple of 128
    Lp = _pack_side(L, lb, p)   # [B, p, D]
    Rp = _pack_side(R, rb, p)

    key = (PAIRS_PER_CORE, p)
    if key not in _NC_CACHE:
        _NC_CACHE[key] = _build_bass(*key)
    nc = _NC_CACHE[key]

    nl = p // 128
    nstat = PAIRS_PER_CORE * nl
    mask_host = np.zeros((128, nstat, 16), np.float16)
    for k in range(nstat):
        mask_host[:, k, k // nl] = 1.0
    in_maps = []
    for c in range(N_CORES):
        Lc = Lp[c * PAIRS_PER_CORE:(c + 1) * PAIRS_PER_CORE]  # [pairs, p, D]
        Rc = Rp[c * PAIRS_PER_CORE:(c + 1) * PAIRS_PER_CORE]
        f16 = np.float16
        nat = lambda X: np.ascontiguousarray(
            X.reshape(PAIRS_PER_CORE, nl, 128, D).transpose(2, 1, 0, 3).astype(f16))
        tr = lambda X: np.ascontiguousarray(
            X.reshape(PAIRS_PER_CORE, p, DC, 128).transpose(3, 2, 0, 1).astype(f16))
        in_maps.append({"l_nat": nat(Lc), "r_nat": nat(Rc),
                        "lt": tr(Lc), "rt": tr(Rc), "mask": mask_host})

    LAST_IN_MAPS.append(in_maps)
    res = run_bass_kernel_spmd(nc, in_maps, list(range(N_CORES)))
    LAST_RESULT = res

    if os.environ.get("KERNEL_BENCH"):
        reps = int(os.environ.get("KERNEL_BENCH_REPS", "20"))
        LAST_TIMING["kernel_wall_s"] = _bench_exec(nc, in_maps, reps)
        LAST_TIMING["overhead_wall_s"] = _noop_baseline(reps)

    out_l = np.concatenate([res.results[c]["out_l"] for c in range(N_CORES)], axis=0)
    out_r = np.concatenate([res.results[c]["out_r"] for c in range(N_CORES)], axis=0)
    return out_l.astype(np.float32), out_r.astype(np.float32)


# revision 26
# speedup vs baseline: 1.0207x; 1.0207x over previous
"""MatchFilter (graph-pair cross-attention + gated segment sum) on 8 trn2 cores.

Math per graph pair b (reference):
    S = L_b @ R_b^T                      [nl, nr]
    P_row = softmax(S, axis=1);  P_col = softmax(S, axis=0)
    wl_i = sigmoid(<L_i, (P_row @ R)_i>) = sigmoid( (sum_j E_ij S_ij) / (sum_j E_ij) )
    wr_j analogously from S^T.
    out_l[b] = sum_i wl_i L_i ;  out_r[b] = sum_j wr_j R_j

Key identity: <L_i, right_atten_i> = sum_j P_row[i,j] * S[i,j] — so the
second attention matmul is never materialized; only row stats of S (and S^T)
are needed.  sigmoid(z) = 0.5 + 0.5*tanh(z/2) keeps ScalarE on one LUT set
(Exp and Tanh share `exp_and_others`).

Sharding: 64 pairs -> 8 cores x 8 pairs, fully local (data parallel over
pairs).  Host pre-swizzles per-core fp16 inputs in both natural ([node, d])
and d-major ([d, node]) layouts so the device does zero transposes.
"""

import os
import numpy as np
from contextlib import ExitStack

import concourse.bass as bass
import concourse.bacc as bacc
import concourse.tile as tile
from concourse import mybir
from concourse.bass_utils import run_bass_kernel_spmd

N_CORES = 8
B = 64            # graph pairs
D = 256           # embedding dim
DC = D // 128     # d-chunks for the 128-deep contraction
PAIRS_PER_CORE = B // N_CORES
# exp(S - EXP_SHIFT): keeps e^S and its row-sums inside f32 range for
# S ~ N(0, 256) (|S| <~ 90).  Max-subtraction is unnecessary because the
# z = t/s ratio is shift-invariant.
EXP_SHIFT = 32.0

LAST_RESULT = None  # BassKernelResults of the most recent run (for test.py)
LAST_TIMING = {}
LAST_IN_MAPS = []

_NC_CACHE = {}


def _build_bass(pairs: int, p: int):
    """Per-core program: `pairs` graph pairs, each padded to `p` nodes/side."""
    f16, f32 = mybir.dt.float16, mybir.dt.float32
    AF = mybir.ActivationFunctionType
    ALU = mybir.AluOpType
    nl = p // 128  # row-chunks per pair side

    nc = bacc.Bacc("TRN2", target_bir_lowering=False, debug=False,
                   num_devices=N_CORES)
    l_nat = nc.dram_tensor("l_nat", [128, nl, pairs, D], f16, kind="ExternalInput").ap()
    r_nat = nc.dram_tensor("r_nat", [128, nl, pairs, D], f16, kind="ExternalInput").ap()
    lt = nc.dram_tensor("lt", [128, DC, pairs, p], f16, kind="ExternalInput").ap()
    rt = nc.dram_tensor("rt", [128, DC, pairs, p], f16, kind="ExternalInput").ap()
    mask = nc.dram_tensor("mask", [128, pairs * (p // 128), 16], f16,
                          kind="ExternalInput").ap()
    out_l = nc.dram_tensor("out_l", [pairs, D], f32, kind="ExternalOutput").ap()
    out_r = nc.dram_tensor("out_r", [pairs, D], f32, kind="ExternalOutput").ap()

    with tile.TileContext(nc) as tc, ExitStack() as ctx:
        sb = ctx.enter_context(tc.tile_pool(name="sb", bufs=1))
        work = ctx.enter_context(tc.tile_pool(name="work", bufs=4))
        stat = ctx.enter_context(tc.tile_pool(name="stat", bufs=1))
        psum = ctx.enter_context(tc.tile_pool(name="psum", bufs=3, space="PSUM"))
        outp = ctx.enter_context(tc.tile_pool(name="outp", bufs=1, space="PSUM"))

        mask_sb = stat.tile([128, pairs * nl, 16], f16, tag="mask_sb")
        l_sb = sb.tile([128, nl, pairs, D], f16, tag="l_sb")
        r_sb = sb.tile([128, nl, pairs, D], f16, tag="r_sb")
        lt_sb = sb.tile([128, DC, pairs, p], f16, tag="lt_sb")
        rt_sb = sb.tile([128, DC, pairs, p], f16, tag="rt_sb")
        # split loads so pair-0 compute can start before the tail arrives;
        # natural-layout tiles after (only needed by the final matmuls)
        nsplit = 2
        step = pairs // nsplit if pairs % nsplit == 0 else pairs
        for s0 in range(0, pairs, step):
            sl = slice(s0, s0 + step)
            nc.sync.dma_start(out=lt_sb[:, :, sl, :], in_=lt[:, :, sl, :])
            nc.sync.dma_start(out=rt_sb[:, :, sl, :], in_=rt[:, :, sl, :])
        nc.sync.dma_start(out=mask_sb, in_=mask)
        # natural-layout loads are only needed by the final matmuls; issue
        # them from the idle GPSIMD (SWDGE) so SP's issue queue stays short
        for s0 in range(0, pairs, step):
            sl = slice(s0, s0 + step)
            nc.gpsimd.dma_start(out=l_sb[:, :, sl, :], in_=l_nat[:, :, sl, :])
            nc.gpsimd.dma_start(out=r_sb[:, :, sl, :], in_=r_nat[:, :, sl, :])

        neg_shift = stat.tile([128, 1], f32, tag="neg_shift")
        nc.vector.memset(neg_shift, -EXP_SHIFT)
        # dummy activation at t=0: pulls the exp_and_others ACT table load
        # off the critical path (it overlaps the input DMA wait)
        warm = stat.tile([128, 1], f32, tag="warm")
        nc.scalar.activation(out=warm, in_=neg_shift, func=AF.Exp,
                             bias=neg_shift, scale=1.0)

        nstat = pairs * nl
        # stats padded to 32B stride so every tiny accum write is 32B-aligned
        sL = stat.tile([128, nstat, 8], f32, tag="sL")
        tL = stat.tile([128, nstat, 8], f32, tag="tL")
        sR = stat.tile([128, nstat, 8], f32, tag="sR")
        tR = stat.tile([128, nstat, 8], f32, tag="tR")

        for b in range(pairs):
            for rc in range(nl):  # row-chunk of the left side (S) / right side (ST)
                k = b * nl + rc
                s_ps = psum.tile([128, p], f32, tag="s")
                st_ps = psum.tile([128, p], f32, tag="st")
                for c in range(DC):
                    nc.tensor.matmul(
                        s_ps, lhsT=lt_sb[:, c, b, rc * 128:(rc + 1) * 128],
                        rhs=rt_sb[:, c, b, :], start=(c == 0), stop=(c == DC - 1))
                for c in range(DC):
                    nc.tensor.matmul(
                        st_ps, lhsT=rt_sb[:, c, b, rc * 128:(rc + 1) * 128],
                        rhs=lt_sb[:, c, b, :], start=(c == 0), stop=(c == DC - 1))
                e = work.tile([128, p], f32, tag="e")
                et = work.tile([128, p], f32, tag="et")
                nc.scalar.activation(out=e, in_=s_ps, func=AF.Exp,
                                     bias=neg_shift, scale=1.0,
                                     accum_out=sL[:, k, 0:1])
                nc.scalar.activation(out=et, in_=st_ps, func=AF.Exp,
                                     bias=neg_shift, scale=1.0,
                                     accum_out=sR[:, k, 0:1])
                scr = work.tile([128, p], f32, tag="scr")
                scrt = work.tile([128, p], f32, tag="scrt")
                nc.vector.scalar_tensor_tensor(
                    out=scr, in0=e, scalar=1.0, in1=s_ps,
                    op0=ALU.mult, op1=ALU.mult, accum_out=tL[:, k, 0:1])
                nc.vector.scalar_tensor_tensor(
                    out=scrt, in0=et, scalar=1.0, in1=st_ps,
                    op0=ALU.mult, op1=ALU.mult, accum_out=tR[:, k, 0:1])

        # gates: w = 0.5 + 0.5*tanh( (t/s) / 2 ) == sigmoid(t/s); the gate for
        # chunk k is splatted onto block-diagonal weights via a host-sent
        # one-hot mask so the final segment sums are [K=128, M=pairs, N=D]
        # accumulating matmuls with output base partition 0.  Processed in two
        # pair-halves so the first half's gate tail overlaps the second
        # half's score/softmax compute.
        outl_ps = outp.tile([pairs, D], f32, tag="outl")
        outr_ps = outp.tile([pairs, D], f32, tag="outr")
        half = (pairs + 1) // 2
        for g0 in range(0, pairs, half):
            g1 = min(g0 + half, pairs)
            k0, k1 = g0 * nl, g1 * nl
            nk = k1 - k0
            for side, s_t, t_t, w_ps, n_sb in (
                    ("l", sL, tL, outl_ps, l_sb), ("r", sR, tR, outr_ps, r_sb)):
                rs = stat.tile([128, nk], f32, tag=f"rs_{side}_{g0}")
                z = stat.tile([128, nk], f32, tag=f"z_{side}_{g0}")
                h = stat.tile([128, nk], f32, tag=f"h_{side}_{g0}")
                w = stat.tile([128, nk], f32, tag=f"w_{side}_{g0}")
                wblk = stat.tile([128, nk, 16], f16, tag=f"wblk_{side}_{g0}")
                nc.vector.reciprocal(out=rs, in_=s_t[:, k0:k1, 0])
                nc.vector.tensor_mul(z, t_t[:, k0:k1, 0], rs)
                nc.scalar.activation(out=h, in_=z, func=AF.Tanh, scale=0.5)
                nc.vector.tensor_scalar(out=w, in0=h, scalar1=0.5, scalar2=0.5,
                                        op0=ALU.mult, op1=ALU.add)
                for kk in range(nk):
                    k = k0 + kk
                    nc.vector.tensor_scalar(
                        out=wblk[:, kk, :], in0=mask_sb[:, k, :],
                        scalar1=w[:, kk:kk + 1], scalar2=None, op0=ALU.mult)
                for kk in range(nk):
                    k = k0 + kk
                    b, rc = k // nl, k % nl
                    nc.tensor.matmul(w_ps, lhsT=wblk[:, kk, 0:pairs],
                                     rhs=n_sb[:, rc, b, :],
                                     start=(k == 0), stop=(k == nstat - 1))

        outl_sb = stat.tile([pairs, D], f32, tag="outl_sb")
        outr_sb = stat.tile([pairs, D], f32, tag="outr_sb")
        nc.vector.tensor_copy(out=outl_sb, in_=outl_ps)
        nc.vector.tensor_copy(out=outr_sb, in_=outr_ps)
        nc.sync.dma_start(out=out_l, in_=outl_sb)
        nc.sync.dma_start(out=out_r, in_=outr_sb)

    nc.compile()
    return nc


def _bench_exec(nc, in_maps, reps):
    """Min wall time of the cached jitted 8-core NEFF dispatch (inputs
    pre-sharded on device; excludes jax tracing and input upload)."""
    import time as _time
    import jax
    import jax.numpy as jnp
    from jax.sharding import Mesh, PartitionSpec, NamedSharding
    from jax.experimental.shard_map import shard_map
    from concourse import bass2jax
    from concourse.bass2jax import _bass_exec_p

    n_cores = len(in_maps)
    part_name = nc.partition_id_tensor.name if nc.partition_id_tensor else None
    in_names, out_names, out_avals = [], [], []
    for alloc in nc.m.functions[0].allocations:
        if not isinstance(alloc, mybir.MemoryLocationSet):
            continue
        name = alloc.memorylocations[0].name
        if alloc.kind == "ExternalInput":
            if name != part_name:
                in_names.append(name)
        elif alloc.kind == "ExternalOutput":
            out_names.append(name)
            out_avals.append(jax.core.ShapedArray(
                tuple(alloc.tensor_shape), mybir.dt.np(alloc.dtype)))
    n_params = len(in_names)
    all_in_names = in_names + out_names
    if part_name is not None:
        all_in_names = all_in_names + [part_name]

    def _body(*args):
        operands = list(args)
        if part_name is not None:
            operands.append(bass2jax.partition_id_tensor())
        return tuple(_bass_exec_p.bind(
            *operands, out_avals=tuple(out_avals), in_names=tuple(all_in_names),
            out_names=tuple(out_names), lowering_input_output_aliases=(),
            sim_require_finite=True, sim_require_nnan=True, nc=nc))

    devices = jax.devices()[:n_cores]
    mesh = Mesh(np.asarray(devices), ("core",))
    spec = PartitionSpec("core")
    fn = jax.jit(shard_map(_body, mesh=mesh,
                           in_specs=(spec,) * (n_params + len(out_names)),
                           out_specs=(spec,) * len(out_names)),
                 keep_unused=True)
    sharding = NamedSharding(mesh, spec)
    dev_ins = [jax.device_put(
        np.concatenate([np.asarray(m[name]) for m in in_maps], axis=0), sharding)
        for name in in_names]
    dev_zeros = [jax.device_put(
        np.zeros((n_cores * a.shape[0], *a.shape[1:]), a.dtype), sharding)
        for a in out_avals]
    fn(*dev_ins, *dev_zeros)[0].block_until_ready()  # warm compile
    best = float("inf")
    for _ in range(reps):
        t0 = _time.perf_counter()
        outs = fn(*dev_ins, *dev_zeros)
        for o in outs:
            o.block_until_ready()
        best = min(best, _time.perf_counter() - t0)
    return best


def _noop_baseline(reps):
    """Min wall time of a near-empty program through the same run path —
    estimates the host/axon dispatch overhead included in kernel_wall_s."""
    import time as _time
    if "noop" not in _NC_CACHE:
        nc = bacc.Bacc("TRN2", target_bir_lowering=False, debug=False,
                       num_devices=N_CORES)
        x = nc.dram_tensor("x", [128, 16], mybir.dt.float32,
                           kind="ExternalInput").ap()
        y = nc.dram_tensor("y", [128, 16], mybir.dt.float32,
                           kind="ExternalOutput").ap()
        with tile.TileContext(nc) as tc, ExitStack() as ctx:
            pool = ctx.enter_context(tc.tile_pool(name="p", bufs=1))
            t = pool.tile([128, 16], mybir.dt.float32)
            nc.sync.dma_start(out=t, in_=x)
            nc.sync.dma_start(out=y, in_=t)
        nc.compile()
        _NC_CACHE["noop"] = nc
    nc = _NC_CACHE["noop"]
    ins = [{"x": np.zeros((128, 16), np.float32)} for _ in range(N_CORES)]
    return _bench_exec(nc, ins, reps)


def sim_time_ns(in_map, pairs, p):
    """CoreSim cost-model time for one core's program (ns)."""
    from concourse import bass_interp
    key = (pairs, p)
    if key not in _NC_CACHE:
        _NC_CACHE[key] = _build_bass(*key)
    sim = bass_interp.CoreSim(_NC_CACHE[key])
    for name, arr in in_map.items():
        sim.tensor(name)[:] = arr
    sim.simulate()
    return int(sim.time)


def _pack_side(emb, seg_id, p):
    """[N, D] ragged -> [B, p, D] zero-padded f32 (no-copy reshape if uniform)."""
    counts = np.bincount(seg_id, minlength=B)
    if (counts == p).all():
        return emb.reshape(B, p, D)
    out = np.zeros((B, p, D), emb.dtype)
    offs = np.concatenate([[0], np.cumsum(counts)])
    for g in range(B):
        out[g, :counts[g]] = emb[offs[g]:offs[g + 1]]
    return out


def kernel(left_graph_emb, right_graph_emb, left_x_batch, right_x_batch):
    global LAST_RESULT
    L = np.ascontiguousarray(np.asarray(left_graph_emb, dtype=np.float32))
    R = np.ascontiguousarray(np.asarray(right_graph_emb, dtype=np.float32))
    lb = np.asarray(left_x_batch).astype(np.int64)
    rb = np.asarray(right_x_batch).astype(np.int64)

    maxseg = max(int(np.bincount(lb, minlength=B).max()),
                 int(np.bincount(rb, minlength=B).max()))
    p = max(128, -(-maxseg // 128) * 128)  # pad width, multiple of 128
    Lp = _pack_side(L, lb, p)   # [B, p, D]
    Rp = _pack_side(R, rb, p)

    key = (PAIRS_PER_CORE, p)
    if key not in _NC_CACHE:
        _NC_CACHE[key] = _build_bass(*key)
    nc = _NC_CACHE[key]

    nl = p // 128
    nstat = PAIRS_PER_CORE * nl
    mask_host = np.zeros((128, nstat, 16), np.float16)
    for k in range(nstat):
        mask_host[:, k, k // nl] = 1.0
    in_maps = []
    for c in range(N_CORES):
        Lc = Lp[c * PAIRS_PER_CORE:(c + 1) * PAIRS_PER_CORE]  # [pairs, p, D]
        Rc = Rp[c * PAIRS_PER_CORE:(c + 1) * PAIRS_PER_CORE]
        f16 = np.float16
        nat = lambda X: np.ascontiguousarray(
            X.reshape(PAIRS_PER_CORE, nl, 128, D).transpose(2, 1, 0, 3).astype(f16))
        tr = lambda X: np.ascontiguousarray(
            X.reshape(PAIRS_PER_CORE, p, DC, 128).transpose(3, 2, 0, 1).astype(f16))
        in_maps.append({"l_nat": nat(Lc), "r_nat": nat(Rc),
                        "lt": tr(Lc), "rt": tr(Rc), "mask": mask_host})

    LAST_IN_MAPS.append(in_maps)
    res = run_bass_kernel_spmd(nc, in_maps, list(range(N_CORES)))
    LAST_RESULT = res

    if os.environ.get("KERNEL_BENCH"):
        reps = int(os.environ.get("KERNEL_BENCH_REPS", "20"))
        LAST_TIMING["kernel_wall_s"] = _bench_exec(nc, in_maps, reps)
        LAST_TIMING["overhead_wall_s"] = _noop_baseline(reps)

    out_l = np.concatenate([res.results[c]["out_l"] for c in range(N_CORES)], axis=0)
    out_r = np.concatenate([res.results[c]["out_r"] for c in range(N_CORES)], axis=0)
    return out_l.astype(np.float32), out_r.astype(np.float32)
